# revision 22
# baseline (speedup 1.0000x reference)
"""Trainium2 Bass kernel: causal multi-head attention (B=4,S=2048,D=1024,H=16).

Sharding (8 cores, host-side pair reduction): core c -> batch b=c//2,
head-half hh=c%2 (local heads hh*8..hh*8+7, i.e. 4 head pairs).  Each core
computes Q/K/V for its 8 heads over ALL 2048 rows, full causal attention,
and a PARTIAL fc_out against the row-shard Wo[hh*512:(hh+1)*512].  The host
sums the two partials per batch (the "all-reduce" of the row-sharded Wo).

Device pipeline per core (all matmuls bf16, f32 accumulation):
  - x^T arrives directly via 8 DMA-xbar transposes from DRAM (no PE
    transposes, no row-major staging).
  - Attention (the ScalarE exp stream is the pacer): per pair g, per
    q-chunk of 512 cols, per k-tile: scores^T pair = two row-tiled
    concurrent matmuls (heads at array rows 0-63 / 64-127) -> one
    1024-wide exp on ScalarE (scale folded, PSUM->SBUF bf16), 0/1 mask
    multiply on diag blocks, ones-augmented AV accumulation one k-step
    behind (row 64 = softmax denominator).
  - All other PE work (V projections, K^T/Q^T of later pairs, fc_out
    tiles) is emitted as "filler" chunks pulled into the exp-wait gaps,
    gated by markers so the in-order PE queue can never deadlock.
  - Finalize per (g, q-chunk): free po via a DVE copy, reciprocal of the
    denominators, GpSimd partition-broadcast, normalize into cat (bf16).
  - fc_out tiles run as filler during the last pair; bf16 output.

Weights are pre-packed on the host into the exact stationary layouts
(bf16).  The program is specialized at build time to the mask's 128x128
block structure (computed from the actual mask input, so it stays correct
for any mask).
"""

import os
import numpy as np
import ml_dtypes

import concourse.bass as bass
import concourse.mybir as mybir
import concourse.tile as tile
from concourse import bacc
from concourse.bass_utils import run_bass_kernel_spmd
from concourse.masks import make_identity

B, S, D, H, HD = 4, 2048, 1024, 16, 64
N_CORES = 8
ST = 128                 # tile edge
NKT = S // ST            # 16 k tiles
NQT = S // ST            # 16 q tiles
NDC = D // ST            # 8 contraction chunks
HL = H // 2              # 8 local heads per core
NG = HL // 2             # 4 local head pairs
NQC = 4                  # q chunks per core
QCW = S // NQC           # 512 cols per q chunk (4 q tiles)
QCT = QCW // ST          # 4 q tiles per chunk

F32 = mybir.dt.float32
BF16 = mybir.dt.bfloat16
BF = ml_dtypes.bfloat16


def _classify(mask: np.ndarray):
    """128x128 block structure of the mask: 0 skip, 1 full, 2 mixed."""
    cls = np.zeros((NQT, NKT), dtype=int)
    for j in range(NQT):
        for k in range(NKT):
            blk = mask[j * ST:(j + 1) * ST, k * ST:(k + 1) * ST]
            if (blk != 0).all():
                cls[j, k] = 1
            elif (blk == 0).all():
                cls[j, k] = 0
            else:
                cls[j, k] = 2
    mixed = [(j, k) for j in range(NQT) for k in range(NKT) if cls[j, k] == 2]
    return cls, mixed


def _runs(valid):
    """Contiguous runs [(ja, jb)] of a sorted list of chunk-local j."""
    runs = []
    for j in valid:
        if runs and j == runs[-1][1] + 1:
            runs[-1][1] = j
        else:
            runs.append([j, j])
    return [(a, b) for a, b in runs]


class Filler:
    """Ordered queue of PE-work chunks with tags (drain points) and gates."""

    def __init__(self):
        self.q = []           # (tag, gate, fn)
        self.open = set()
        self.emitted = set()

    def add(self, fn, tag=None, gate=None):
        self.q.append((tag, gate, fn))

    def open_gate(self, gate):
        self.open.add(gate)

    def _emit_front(self):
        tag, gate, fn = self.q.pop(0)
        fn()
        if tag:
            self.emitted.add(tag)
        return tag

    def pull(self, n=1):
        for _ in range(n):
            if not self.q:
                return
            tag, gate, fn = self.q[0]
            if gate is not None and gate not in self.open:
                return
            self._emit_front()

    def drain(self, tag):
        if tag in self.emitted:
            return
        while self.q:
            g = self.q[0][1]
            assert g is None or g in self.open, f"drain past closed gate {g}"
            if self._emit_front() == tag:
                return
        raise KeyError(tag)

    def drain_all(self):
        while self.q:
            self._emit_front()


def _build(cls, mixed, n_maskt):
    nc = bacc.Bacc("TRN2", target_bir_lowering=False, debug=False,
                   num_devices=N_CORES)

    x_d = nc.dram_tensor("x", [S, D], BF16, kind="ExternalInput")
    wqp_d = nc.dram_tensor("wqp", [ST, NDC, NG, ST], BF16, kind="ExternalInput")
    wkp_d = nc.dram_tensor("wkp", [ST, NDC, NG, ST], BF16, kind="ExternalInput")
    wvb_d = nc.dram_tensor("wvb", [ST, NDC, HL * HD], BF16, kind="ExternalInput")
    wob_d = nc.dram_tensor("wob", [ST, NG, D], BF16, kind="ExternalInput")
    bqp_d = nc.dram_tensor("bqp", [ST, NG], F32, kind="ExternalInput")
    bkp_d = nc.dram_tensor("bkp", [ST, NG], F32, kind="ExternalInput")
    bvf_d = nc.dram_tensor("bvf", [HL, HD], F32, kind="ExternalInput")
    bob_d = nc.dram_tensor("bob", [D], F32, kind="ExternalInput")
    mt_d = nc.dram_tensor("maskt", [n_maskt, ST, ST], BF16, kind="ExternalInput")
    out_d = nc.dram_tensor("out", [S, D], BF16, kind="ExternalOutput")

    mixed_idx = {jk: i for i, jk in enumerate(mixed)}

    chunk_ks, chunk_vj = [], []
    for qc in range(NQC):
        vj = {}
        for k in range(NKT):
            v = [j for j in range(QCT) if cls[qc * QCT + j, k]]
            if v:
                vj[k] = v
        chunk_ks.append(sorted(vj))
        chunk_vj.append(vj)

    with tile.TileContext(nc) as tc:
        with tc.tile_pool(name="pp", bufs=1) as pp:
            # ---- persistent SBUF ----------------------------------------
            kt = [pp.tile([ST, S], BF16, name=f"kt{g}", tag=f"kt{g}")
                  for g in range(NG)]
            qt = [pp.tile([ST, S], BF16, name=f"qt{g}", tag=f"qt{g}")
                  for g in range(NG)]
            cat = [pp.tile([ST, S], BF16, name=f"cat{g}", tag=f"cat{g}")
                   for g in range(NG)]
            xt = [pp.tile([ST, S], BF16, name=f"xt{c}", tag=f"xt{c}")
                  for c in range(NDC)]
            vb = pp.tile([ST, NKT, HL, HD + 1], BF16, name="vb", tag="vb")
            wqp = pp.tile([ST, NDC, NG, ST], BF16, name="wqp", tag="wqp")
            wkp = pp.tile([ST, NDC, NG, ST], BF16, name="wkp", tag="wkp")
            wvb = pp.tile([ST, NDC, HL * HD], BF16, name="wvb", tag="wvb")
            wob = pp.tile([ST, NG, D], BF16, name="wob", tag="wob")
            bqp = pp.tile([ST, NG], F32, name="bqp", tag="bqp")
            bkp = pp.tile([ST, NG], F32, name="bkp", tag="bkp")
            bvf = pp.tile([ST, HL, HD], F32, name="bvf", tag="bvf")
            bob = pp.tile([ST, D], F32, name="bob", tag="bob")
            mtb = pp.tile([ST, max(n_maskt, 1), ST], BF16, name="mtb", tag="mtb")
            ident = pp.tile([ST, ST], BF16, name="ident", tag="ident")

            make_identity(nc, ident[:, :])
            # weights on the gpsimd (SWDGE) queue, most-urgent first
            nc.gpsimd.dma_start(wkp[:, :, :, :], wkp_d.ap())
            nc.gpsimd.dma_start(wvb[:, :, :], wvb_d.ap())
            nc.gpsimd.dma_start(wqp[:, :, :, :], wqp_d.ap())
            nc.gpsimd.dma_start(mtb[:, :, :],
                                mt_d.ap().rearrange("m p f -> p m f"))
            nc.gpsimd.dma_start(wob[:, :, :], wob_d.ap())
            # small tensors after the x chunks on the HWDGE queues
            nc.scalar.dma_start(bqp[:, :], bqp_d.ap())
            nc.scalar.dma_start(bkp[:, :], bkp_d.ap())
            src = bvf_d.ap()
            nc.scalar.dma_start(
                bvf[:, :, :],
                bass.AP(tensor=src.tensor, offset=src.offset,
                        ap=[[0, ST]] + list(src.ap)))
            src = bob_d.ap()
            nc.scalar.dma_start(
                bob[:, :],
                bass.AP(tensor=src.tensor, offset=src.offset,
                        ap=[[0, ST]] + list(src.ap)))

            nc.vector.memset(vb[:, :, :, HD:HD + 1], 1.0)

            # ---- ramp: x^T(st0-3) via PE transposes, K0/Q0 sg0, V st0-3
            pxb_cm = tc.tile_pool(name="pxb", bufs=4)
            pxb = pxb_cm.__enter__()
            with (
                tc.tile_pool(name="ppst", bufs=3, space="PSUM") as ppst,
                tc.tile_pool(name="ppvr", bufs=2, space="PSUM") as ppvr,
            ):
                def emit_v_ramp(st):
                    psv = ppvr.tile([ST, HL * HD], F32, tag="pvr")
                    for c in range(NDC):
                        nc.tensor.matmul(
                            psv[:, :], xt[c][:, st * ST:(st + 1) * ST],
                            wvb[:, c, :], start=(c == 0), stop=(c == NDC - 1),
                            skip_group_check=True)
                    nc.vector.tensor_add(
                        vb[:, st, :, 0:HD],
                        psv[:, :].rearrange("p (h e) -> p h e", h=HL),
                        bvf[:, :, :])

                def emit_kq_ramp(g, sg, which):
                    w_t, bias_t, dst = ((wkp, bkp, kt[g]) if which == 0
                                        else (wqp, bqp, qt[g]))
                    ps = ppvr.tile([ST, 512], F32, tag="pvr")
                    for c in range(NDC):
                        nc.tensor.matmul(
                            ps[:, :], w_t[:, c, g, :],
                            xt[c][:, sg * 512:(sg + 1) * 512],
                            start=(c == 0), stop=(c == NDC - 1),
                            skip_group_check=True)
                    nc.vector.tensor_scalar(
                        dst[:, sg * 512:(sg + 1) * 512], ps[:, :],
                        bias_t[:, g:g + 1], None, mybir.AluOpType.add)
                # (sg0 of pair 0 is emitted here in the ramp)

                for st in range(4):
                    xb = pxb.tile([ST, D], BF16, tag="xb")
                    eng = nc.sync if st % 2 == 0 else nc.scalar
                    eng.dma_start(xb[:, :],
                                  x_d.ap()[st * ST:(st + 1) * ST, :])
                    for c in range(NDC):
                        pst = ppst.tile([ST, ST], BF16, tag="pst")
                        nc.tensor.transpose(
                            pst[:, :], xb[:, c * ST:(c + 1) * ST], ident[:, :])
                        nc.scalar.copy(xt[c][:, st * ST:(st + 1) * ST],
                                       pst[:, :])
                emit_kq_ramp(0, 0, 0)
                emit_kq_ramp(0, 0, 1)
                for s0 in range(4):
                    emit_v_ramp(s0)

            with (
                tc.tile_pool(name="ppsc", bufs=2, space="PSUM") as ppsc,
                tc.tile_pool(name="ppo", bufs=1, space="PSUM") as ppo,
                tc.tile_pool(name="ppv", bufs=2, space="PSUM") as ppv,
                tc.tile_pool(name="ppt", bufs=3) as ppt,
                tc.tile_pool(name="pfin", bufs=2) as pfin,
                tc.tile_pool(name="pfcs", bufs=3) as pfcs,
            ):
                def emit_xt(st):
                    xb = pxb.tile([ST, D], BF16, tag="xb")
                    eng = nc.sync if st % 2 == 0 else nc.scalar
                    eng.dma_start(xb[:, :],
                                  x_d.ap()[st * ST:(st + 1) * ST, :])
                    for c in range(NDC):
                        pst = ppv.tile([ST, ST], BF16, tag="pv", name="pst")
                        nc.tensor.transpose(
                            pst[:, :], xb[:, c * ST:(c + 1) * ST], ident[:, :])
                        nc.vector.tensor_copy(
                            xt[c][:, st * ST:(st + 1) * ST], pst[:, :])

                def emit_v(st):
                    psv = ppv.tile([ST, HL * HD], F32, tag="pv")
                    for c in range(NDC):
                        nc.tensor.matmul(
                            psv[:, :], xt[c][:, st * ST:(st + 1) * ST],
                            wvb[:, c, :], start=(c == 0), stop=(c == NDC - 1),
                            skip_group_check=True)
                    nc.vector.tensor_add(
                        vb[:, st, :, 0:HD],
                        psv[:, :].rearrange("p (h e) -> p h e", h=HL),
                        bvf[:, :, :])

                def emit_kq(g, sg, which):
                    w_t, bias_t, dst = ((wkp, bkp, kt[g]) if which == 0
                                        else (wqp, bqp, qt[g]))
                    ps = ppv.tile([ST, 512], F32, tag="pv")
                    for c in range(NDC):
                        nc.tensor.matmul(
                            ps[:, :], w_t[:, c, g, :],
                            xt[c][:, sg * 512:(sg + 1) * 512],
                            start=(c == 0), stop=(c == NDC - 1),
                            skip_group_check=True)
                    nc.vector.tensor_scalar(
                        dst[:, sg * 512:(sg + 1) * 512], ps[:, :],
                        bias_t[:, g:g + 1], None, mybir.AluOpType.add)

                def emit_fc(jt):
                    py = [ppv.tile([ST, 512], F32, tag="pv", name=f"py{n}")
                          for n in range(2)]
                    for g in range(NG):
                        for n in range(2):
                            nc.tensor.matmul(
                                py[n][:, :],
                                cat[g][:, jt * ST:(jt + 1) * ST],
                                wob[:, g, n * 512:(n + 1) * 512],
                                start=(g == 0), stop=(g == NG - 1),
                                skip_group_check=True)
                    ysb = pfcs.tile([ST, D], BF16, tag="ysb")
                    for n in range(2):
                        nc.vector.tensor_add(ysb[:, n * 512:(n + 1) * 512],
                                             py[n][:, :],
                                             bob[:, n * 512:(n + 1) * 512])
                    eng = nc.sync if jt % 2 == 0 else nc.scalar
                    eng.dma_start(out_d.ap()[jt * ST:(jt + 1) * ST, :],
                                  ysb[:, :])

                # ---- filler queue --------------------------------------
                fil = Filler()
                for blk in range(1, 4):
                    for st in range(4 * blk, 4 * blk + 4):
                        fil.add(lambda st=st: emit_xt(st), tag=f"xt{st}")
                        fil.add(lambda st=st: emit_v(st), tag=f"v{st}")
                    fil.add(lambda blk=blk: emit_kq(0, blk, 0))
                    fil.add(lambda blk=blk: emit_kq(0, blk, 1),
                            tag=f"kq0s{blk}")
                for g in range(1, NG):
                    for sg in range(4):
                        fil.add(lambda g=g, sg=sg: emit_kq(g, sg, 0))
                        fil.add(lambda g=g, sg=sg: emit_kq(g, sg, 1))
                    fil.add(lambda: None, tag=f"pair{g}")
                for qcf in (1, 2, 3, 0):
                    for jt in range(qcf * QCT, (qcf + 1) * QCT):
                        fil.add(lambda jt=jt: emit_fc(jt), tag=f"fc{jt}",
                                gate=f"cat_qc{jt // QCT}")

                # ---- attention (exp-paced), filler in the gaps ---------
                for g in range(NG):
                    if g > 0:
                        fil.drain(f"pair{g}")
                    qcs = (1, 2, 3, 0) if g == NG - 1 else range(NQC)
                    for qc in qcs:
                        if g == 0 and qc > 0:
                            fil.drain(f"kq0s{qc}")
                        ks = chunk_ks[qc]
                        vjm = chunk_vj[qc]
                        if not ks:
                            continue
                        union = sorted({j for v in vjm.values() for j in v})
                        fast = vjm[ks[0]] == union
                        po = ppo.tile([HD + 1, 2 * QCW], F32, tag="po")
                        if not fast:
                            nc.vector.memset(po[:, :], 0.0)
                        nks = len(ks)

                        def emit_av(k, idx, runs, pt):
                            for h in range(2):
                                for ja, jb in runs:
                                    nc.tensor.matmul(
                                        po[0:HD + 1,
                                           h * QCW + ja * ST:
                                           h * QCW + (jb + 1) * ST],
                                        vb[:, k, 2 * g + h, :],
                                        pt[:, h * QCW + ja * ST:
                                           h * QCW + (jb + 1) * ST],
                                        start=(fast and idx == 0),
                                        stop=(fast and idx == nks - 1),
                                        skip_group_check=True)

                        pending = None
                        for idx, k in enumerate(ks):
                            runs = _runs(vjm[k])
                            psc = ppsc.tile([ST, 2 * QCW], F32, tag="psc")
                            for ja, jb in runs:
                                for h in range(2):
                                    nc.tensor.matmul(
                                        psc[:, h * QCW + ja * ST:
                                            h * QCW + (jb + 1) * ST],
                                        kt[g][h * HD:(h + 1) * HD,
                                              k * ST:(k + 1) * ST],
                                        qt[g][h * HD:(h + 1) * HD,
                                              qc * QCW + ja * ST:
                                              qc * QCW + (jb + 1) * ST],
                                        start=True, stop=True)
                            if pending is not None:
                                emit_av(*pending)
                            pt = ppt.tile([ST, 2 * QCW], BF16, tag="pt")
                            nc.scalar.activation(
                                pt[:, :], psc[:, :],
                                mybir.ActivationFunctionType.Exp,
                                scale=1.0 / float(np.sqrt(HD)))
                            for j in vjm[k]:
                                if cls[qc * QCT + j, k] == 2:
                                    m = mixed_idx[(qc * QCT + j, k)]
                                    for h in range(2):
                                        nc.vector.tensor_mul(
                                            pt[:, h * QCW + j * ST:
                                               h * QCW + (j + 1) * ST],
                                            pt[:, h * QCW + j * ST:
                                               h * QCW + (j + 1) * ST],
                                            mtb[:, m, :])
                            pending = (k, idx, runs, pt)
                            fil.pull(1)
                        emit_av(*pending)
                        # finalize (g, qc): free po via a DVE copy, then
                        # normalize out of SBUF.
                        sfin = pfin.tile([HD + 1, 2 * QCW], F32, tag="sfin")
                        nc.vector.tensor_copy(sfin[:, :], po[:, :])
                        ltmp = pfin.tile([1, 2 * QCW], F32, tag="ltmp")
                        nc.vector.tensor_copy(ltmp[:, :], sfin[HD:HD + 1, :])
                        rec = pfin.tile([1, 2 * QCW], F32, tag="rec")
                        nc.vector.reciprocal_approx_fast(rec[:, :], ltmp[:, :])
                        rbs = pfin.tile([HD, 2 * QCW], F32, tag="rbs")
                        nc.gpsimd.partition_broadcast(
                            rbs[:, :], rec[0:1, :], channels=HD)
                        for h in range(2):
                            nc.vector.tensor_mul(
                                cat[g][h * HD:(h + 1) * HD,
                                       qc * QCW:(qc + 1) * QCW],
                                sfin[0:HD, h * QCW:(h + 1) * QCW],
                                rbs[:, h * QCW:(h + 1) * QCW])
                        if g == NG - 1:
                            fil.open_gate(f"cat_qc{qc}")
                fil.drain_all()
            pxb_cm.__exit__(None, None, None)

    nc.compile()
    return nc


_CACHE = {}
LAST_RESULT = None


def _get_program(mask):
    key = mask.tobytes()
    if key not in _CACHE:
        cls, mixed = _classify(mask)
        _CACHE[key] = (_build(cls, mixed, max(len(mixed), 1)), cls, mixed)
    return _CACHE[key]


def kernel(x, mask, Wq, bq, Wk, bk, Wv, bv, Wo, bo):
    x = np.asarray(x, dtype=np.float32)
    mask = np.asarray(mask)
    Wq = np.asarray(Wq, dtype=np.float32)
    Wk = np.asarray(Wk, dtype=np.float32)
    Wv = np.asarray(Wv, dtype=np.float32)
    Wo = np.asarray(Wo, dtype=np.float32)
    nc, cls, mixed = _get_program(mask)

    n_maskt = max(len(mixed), 1)
    mt = np.zeros((n_maskt, ST, ST), dtype=BF)
    for i, (j, k) in enumerate(mixed):
        blk = mask[j * ST:(j + 1) * ST, k * ST:(k + 1) * ST]
        mt[i] = (blk != 0).T.astype(BF)

    def pack_pair(W, hh):
        # [128, NDC, NG, 128]: [p, c, g, m*64+e] = W[8hh + 2g+m, 128c+p, e]
        Wl = W[hh * HL:(hh + 1) * HL].reshape(NG, 2, NDC, ST, HD)
        return np.ascontiguousarray(
            Wl.transpose(3, 2, 0, 1, 4).reshape(ST, NDC, NG, ST).astype(BF))

    in_maps = []
    for c in range(N_CORES):
        b, hh = c // 2, c % 2
        Wvl = Wv[hh * HL:(hh + 1) * HL].reshape(HL, NDC, ST, HD)
        wvb = np.ascontiguousarray(
            Wvl.transpose(2, 1, 0, 3).reshape(ST, NDC, HL * HD).astype(BF))
        Wol = Wo[hh * HL * HD:(hh + 1) * HL * HD].reshape(NG, 2, HD, D)
        wob = np.ascontiguousarray(
            Wol.transpose(1, 2, 0, 3).reshape(ST, NG, D).astype(BF))
        bql = np.asarray(bq, dtype=np.float32)[hh * HL:(hh + 1) * HL]
        bkl = np.asarray(bk, dtype=np.float32)[hh * HL:(hh + 1) * HL]
        bqp = np.ascontiguousarray(
            bql.reshape(NG, 2, HD).transpose(1, 2, 0).reshape(ST, NG))
        bkp = np.ascontiguousarray(
            bkl.reshape(NG, 2, HD).transpose(1, 2, 0).reshape(ST, NG))
        m = {
            "x": np.ascontiguousarray(x[b].astype(BF)),
            "wqp": pack_pair(Wq, hh),
            "wkp": pack_pair(Wk, hh),
            "wvb": wvb,
            "wob": wob,
            "bqp": bqp,
            "bkp": bkp,
            "bvf": np.ascontiguousarray(
                np.asarray(bv, dtype=np.float32)[hh * HL:(hh + 1) * HL]),
            "bob": (np.asarray(bo, dtype=np.float32) if hh == 0
                    else np.zeros(D, dtype=np.float32)),
            "maskt": mt,
        }
        in_maps.append(m)

    res = run_bass_kernel_spmd(
        nc, in_maps, core_ids=list(range(N_CORES)),
        trace=os.environ.get("BASS_KERNEL_TRACE", "0") == "1")
    global LAST_RESULT
    LAST_RESULT = res

    out = np.empty((B, S, D), dtype=np.float32)
    for b in range(B):
        out[b] = (res.results[2 * b]["out"].astype(np.float32)
                  + res.results[2 * b + 1]["out"].astype(np.float32))
    return out


# revision 23
# speedup vs baseline: 1.2077x; 1.2077x over previous
"""Trainium2 Bass kernel: causal multi-head attention (B=4,S=2048,D=1024,H=16).

Sharding (8 cores, host-side pair reduction): core c -> batch b=c//2,
head-half hh=c%2 (local heads hh*8..hh*8+7, i.e. 4 head pairs).  Each core
computes Q/K/V for its 8 heads over ALL 2048 rows, full causal attention,
and a PARTIAL fc_out against the row-shard Wo[hh*512:(hh+1)*512].  The host
sums the two partials per batch (the "all-reduce" of the row-sharded Wo).

Device pipeline per core (all matmuls bf16, f32 accumulation):
  - x^T arrives directly via 8 DMA-xbar transposes from DRAM (no PE
    transposes, no row-major staging).
  - Attention (the ScalarE exp stream is the pacer): per pair g, per
    q-chunk of 512 cols, per k-tile: scores^T pair = two row-tiled
    concurrent matmuls (heads at array rows 0-63 / 64-127) -> one
    1024-wide exp on ScalarE (scale folded, PSUM->SBUF bf16), 0/1 mask
    multiply on diag blocks, ones-augmented AV accumulation one k-step
    behind (row 64 = softmax denominator).
  - All other PE work (V projections, K^T/Q^T of later pairs, fc_out
    tiles) is emitted as "filler" chunks pulled into the exp-wait gaps,
    gated by markers so the in-order PE queue can never deadlock.
  - Finalize per (g, q-chunk): free po via a DVE copy, reciprocal of the
    denominators, GpSimd partition-broadcast, normalize into cat (bf16).
  - fc_out tiles run as filler during the last pair; bf16 output.

Weights are pre-packed on the host into the exact stationary layouts
(bf16).  The program is specialized at build time to the mask's 128x128
block structure (computed from the actual mask input, so it stays correct
for any mask).
"""

import os
import numpy as np
import ml_dtypes

import concourse.bass as bass
import concourse.mybir as mybir
import concourse.tile as tile
from concourse import bacc
from concourse.bass_utils import run_bass_kernel_spmd
from concourse.masks import make_identity

B, S, D, H, HD = 4, 2048, 1024, 16, 64
N_CORES = 8
ST = 128                 # tile edge
NKT = S // ST            # 16 k tiles
NQT = S // ST            # 16 q tiles
NDC = D // ST            # 8 contraction chunks
HL = H // 2              # 8 local heads per core
NG = HL // 2             # 4 local head pairs
NQC = 4                  # q chunks per core
QCW = S // NQC           # 512 cols per q chunk (4 q tiles)
QCT = QCW // ST          # 4 q tiles per chunk

F32 = mybir.dt.float32
BF16 = mybir.dt.bfloat16
BF = ml_dtypes.bfloat16


def _classify(mask: np.ndarray):
    """128x128 block structure of the mask: 0 skip, 1 full, 2 mixed."""
    cls = np.zeros((NQT, NKT), dtype=int)
    for j in range(NQT):
        for k in range(NKT):
            blk = mask[j * ST:(j + 1) * ST, k * ST:(k + 1) * ST]
            if (blk != 0).all():
                cls[j, k] = 1
            elif (blk == 0).all():
                cls[j, k] = 0
            else:
                cls[j, k] = 2
    mixed = [(j, k) for j in range(NQT) for k in range(NKT) if cls[j, k] == 2]
    return cls, mixed


def _runs(valid):
    """Contiguous runs [(ja, jb)] of a sorted list of chunk-local j."""
    runs = []
    for j in valid:
        if runs and j == runs[-1][1] + 1:
            runs[-1][1] = j
        else:
            runs.append([j, j])
    return [(a, b) for a, b in runs]


class Filler:
    """Ordered queue of PE-work chunks with tags (drain points) and gates."""

    def __init__(self):
        self.q = []           # (tag, gate, fn)
        self.open = set()
        self.emitted = set()

    def add(self, fn, tag=None, gate=None):
        self.q.append((tag, gate, fn))

    def open_gate(self, gate):
        self.open.add(gate)

    def _emit_front(self):
        tag, gate, fn = self.q.pop(0)
        fn()
        if tag:
            self.emitted.add(tag)
        return tag

    def pull(self, n=1):
        for _ in range(n):
            if not self.q:
                return
            tag, gate, fn = self.q[0]
            if gate is not None and gate not in self.open:
                return
            self._emit_front()

    def drain(self, tag):
        if tag in self.emitted:
            return
        while self.q:
            g = self.q[0][1]
            assert g is None or g in self.open, f"drain past closed gate {g}"
            if self._emit_front() == tag:
                return
        raise KeyError(tag)

    def drain_all(self):
        while self.q:
            self._emit_front()


def _build(cls, mixed, n_maskt):
    nc = bacc.Bacc("TRN2", target_bir_lowering=False, debug=False,
                   num_devices=N_CORES)

    x_d = nc.dram_tensor("x", [S, D], BF16, kind="ExternalInput")
    wqp_d = nc.dram_tensor("wqp", [ST, NDC, NG, ST], BF16, kind="ExternalInput")
    wkp_d = nc.dram_tensor("wkp", [ST, NDC, NG, ST], BF16, kind="ExternalInput")
    wvb_d = nc.dram_tensor("wvb", [ST, NDC, HL * HD], BF16, kind="ExternalInput")
    wob_d = nc.dram_tensor("wob", [ST, NG, D], BF16, kind="ExternalInput")
    bqp_d = nc.dram_tensor("bqp", [ST, NG], F32, kind="ExternalInput")
    bkp_d = nc.dram_tensor("bkp", [ST, NG], F32, kind="ExternalInput")
    bvf_d = nc.dram_tensor("bvf", [HL, HD], F32, kind="ExternalInput")
    bob_d = nc.dram_tensor("bob", [D], F32, kind="ExternalInput")
    mt_d = nc.dram_tensor("maskt", [n_maskt, ST, ST], BF16, kind="ExternalInput")
    out_d = nc.dram_tensor("out", [S, D], BF16, kind="ExternalOutput")

    mixed_idx = {jk: i for i, jk in enumerate(mixed)}

    chunk_ks, chunk_vj = [], []
    for qc in range(NQC):
        vj = {}
        for k in range(NKT):
            v = [j for j in range(QCT) if cls[qc * QCT + j, k]]
            if v:
                vj[k] = v
        chunk_ks.append(sorted(vj))
        chunk_vj.append(vj)

    with tile.TileContext(nc) as tc:
        with tc.tile_pool(name="pp", bufs=1) as pp:
            # ---- persistent SBUF ----------------------------------------
            kt = [pp.tile([ST, S], BF16, name=f"kt{g}", tag=f"kt{g}")
                  for g in range(NG)]
            qt = [pp.tile([ST, S], BF16, name=f"qt{g}", tag=f"qt{g}")
                  for g in range(NG)]
            cat = [pp.tile([ST, S], BF16, name=f"cat{g}", tag=f"cat{g}")
                   for g in range(NG)]
            xt = [pp.tile([ST, S], BF16, name=f"xt{c}", tag=f"xt{c}")
                  for c in range(NDC)]
            vb = pp.tile([ST, NKT, HL, HD + 1], BF16, name="vb", tag="vb")
            wqp = pp.tile([ST, NDC, NG, ST], BF16, name="wqp", tag="wqp")
            wkp = pp.tile([ST, NDC, NG, ST], BF16, name="wkp", tag="wkp")
            wvb = pp.tile([ST, NDC, HL * HD], BF16, name="wvb", tag="wvb")
            wob = pp.tile([ST, NG, D], BF16, name="wob", tag="wob")
            bqp = pp.tile([ST, NG], F32, name="bqp", tag="bqp")
            bkp = pp.tile([ST, NG], F32, name="bkp", tag="bkp")
            bvf = pp.tile([ST, HL, HD], F32, name="bvf", tag="bvf")
            bob = pp.tile([ST, D], F32, name="bob", tag="bob")
            mtb = pp.tile([ST, max(n_maskt, 1), ST], BF16, name="mtb", tag="mtb")
            ident = pp.tile([ST, ST], BF16, name="ident", tag="ident")

            # prefetch the exp ACT table set during the DMA-bound ramp so
            # the first real exp doesn't pay the ~2.7us table load.
            actwarm = pp.tile([1, 16], F32, name="actwarm", tag="actwarm")
            nc.vector.memset(actwarm[:, :], 0.0)
            nc.scalar.activation(actwarm[:, :], actwarm[:, :],
                                 mybir.ActivationFunctionType.Exp)
            make_identity(nc, ident[:, :])
            # weights on the gpsimd (SWDGE) queue, most-urgent first
            nc.gpsimd.dma_start(wkp[:, :, :, :], wkp_d.ap())
            nc.gpsimd.dma_start(wvb[:, :, :], wvb_d.ap())
            nc.gpsimd.dma_start(wqp[:, :, :, :], wqp_d.ap())
            nc.gpsimd.dma_start(mtb[:, :, :],
                                mt_d.ap().rearrange("m p f -> p m f"))
            nc.gpsimd.dma_start(wob[:, :, :], wob_d.ap())
            # small tensors after the x chunks on the HWDGE queues
            nc.scalar.dma_start(bqp[:, :], bqp_d.ap())
            nc.scalar.dma_start(bkp[:, :], bkp_d.ap())
            src = bvf_d.ap()
            nc.scalar.dma_start(
                bvf[:, :, :],
                bass.AP(tensor=src.tensor, offset=src.offset,
                        ap=[[0, ST]] + list(src.ap)))
            src = bob_d.ap()
            nc.scalar.dma_start(
                bob[:, :],
                bass.AP(tensor=src.tensor, offset=src.offset,
                        ap=[[0, ST]] + list(src.ap)))

            nc.vector.memset(vb[:, :, :, HD:HD + 1], 1.0)

            # ---- ramp: x^T(st0-3) via PE transposes, K0/Q0 sg0, V st0-3
            pxb_cm = tc.tile_pool(name="pxb", bufs=4)
            pxb = pxb_cm.__enter__()
            with (
                tc.tile_pool(name="ppst", bufs=3, space="PSUM") as ppst,
                tc.tile_pool(name="ppvr", bufs=2, space="PSUM") as ppvr,
            ):
                def emit_v_ramp(st):
                    psv = ppvr.tile([ST, HL * HD], F32, tag="pvr")
                    for c in range(NDC):
                        nc.tensor.matmul(
                            psv[:, :], xt[c][:, st * ST:(st + 1) * ST],
                            wvb[:, c, :], start=(c == 0), stop=(c == NDC - 1),
                            skip_group_check=True)
                    nc.vector.tensor_add(
                        vb[:, st, :, 0:HD],
                        psv[:, :].rearrange("p (h e) -> p h e", h=HL),
                        bvf[:, :, :])

                def emit_kq_ramp(g, sg, which):
                    w_t, bias_t, dst = ((wkp, bkp, kt[g]) if which == 0
                                        else (wqp, bqp, qt[g]))
                    ps = ppvr.tile([ST, 512], F32, tag="pvr")
                    for c in range(NDC):
                        nc.tensor.matmul(
                            ps[:, :], w_t[:, c, g, :],
                            xt[c][:, sg * 512:(sg + 1) * 512],
                            start=(c == 0), stop=(c == NDC - 1),
                            skip_group_check=True)
                    nc.vector.tensor_scalar(
                        dst[:, sg * 512:(sg + 1) * 512], ps[:, :],
                        bias_t[:, g:g + 1], None, mybir.AluOpType.add)
                # (sg0 of pair 0 is emitted here in the ramp)

                for st in range(4):
                    xb = pxb.tile([ST, D], BF16, tag="xb")
                    eng = nc.sync if st % 2 == 0 else nc.scalar
                    eng.dma_start(xb[:, :],
                                  x_d.ap()[st * ST:(st + 1) * ST, :])
                    for c in range(NDC):
                        pst = ppst.tile([ST, ST], BF16, tag="pst")
                        nc.tensor.transpose(
                            pst[:, :], xb[:, c * ST:(c + 1) * ST], ident[:, :])
                        nc.scalar.copy(xt[c][:, st * ST:(st + 1) * ST],
                                       pst[:, :])
                emit_kq_ramp(0, 0, 0)
                emit_kq_ramp(0, 0, 1)
                for s0 in range(4):
                    emit_v_ramp(s0)

            with (
                tc.tile_pool(name="ppsc", bufs=2, space="PSUM") as ppsc,
                tc.tile_pool(name="ppo", bufs=1, space="PSUM") as ppo,
                tc.tile_pool(name="ppv", bufs=2, space="PSUM") as ppv,
                tc.tile_pool(name="ppt", bufs=3) as ppt,
                tc.tile_pool(name="pfin", bufs=2) as pfin,
                tc.tile_pool(name="pfcs", bufs=3) as pfcs,
            ):
                def emit_xt(st):
                    xb = pxb.tile([ST, D], BF16, tag="xb")
                    eng = nc.sync if st % 2 == 0 else nc.scalar
                    eng.dma_start(xb[:, :],
                                  x_d.ap()[st * ST:(st + 1) * ST, :])
                    for c in range(NDC):
                        pst = ppv.tile([ST, ST], BF16, tag="pv", name="pst")
                        nc.tensor.transpose(
                            pst[:, :], xb[:, c * ST:(c + 1) * ST], ident[:, :])
                        nc.vector.tensor_copy(
                            xt[c][:, st * ST:(st + 1) * ST], pst[:, :])

                def emit_v(st):
                    psv = ppv.tile([ST, HL * HD], F32, tag="pv")
                    for c in range(NDC):
                        nc.tensor.matmul(
                            psv[:, :], xt[c][:, st * ST:(st + 1) * ST],
                            wvb[:, c, :], start=(c == 0), stop=(c == NDC - 1),
                            skip_group_check=True)
                    nc.vector.tensor_add(
                        vb[:, st, :, 0:HD],
                        psv[:, :].rearrange("p (h e) -> p h e", h=HL),
                        bvf[:, :, :])

                def emit_kq(g, sg, which):
                    w_t, bias_t, dst = ((wkp, bkp, kt[g]) if which == 0
                                        else (wqp, bqp, qt[g]))
                    ps = ppv.tile([ST, 512], F32, tag="pv")
                    for c in range(NDC):
                        nc.tensor.matmul(
                            ps[:, :], w_t[:, c, g, :],
                            xt[c][:, sg * 512:(sg + 1) * 512],
                            start=(c == 0), stop=(c == NDC - 1),
                            skip_group_check=True)
                    nc.vector.tensor_scalar(
                        dst[:, sg * 512:(sg + 1) * 512], ps[:, :],
                        bias_t[:, g:g + 1], None, mybir.AluOpType.add)

                def emit_fc(jt):
                    py = [ppv.tile([ST, 512], F32, tag="pv", name=f"py{n}")
                          for n in range(2)]
                    for g in range(NG):
                        for n in range(2):
                            nc.tensor.matmul(
                                py[n][:, :],
                                cat[g][:, jt * ST:(jt + 1) * ST],
                                wob[:, g, n * 512:(n + 1) * 512],
                                start=(g == 0), stop=(g == NG - 1),
                                skip_group_check=True)
                    ysb = pfcs.tile([ST, D], BF16, tag="ysb")
                    for n in range(2):
                        nc.vector.tensor_add(ysb[:, n * 512:(n + 1) * 512],
                                             py[n][:, :],
                                             bob[:, n * 512:(n + 1) * 512])
                    eng = nc.sync if jt % 2 == 0 else nc.scalar
                    eng.dma_start(out_d.ap()[jt * ST:(jt + 1) * ST, :],
                                  ysb[:, :])

                # ---- filler queue --------------------------------------
                fil = Filler()
                for blk in range(1, 4):
                    for st in range(4 * blk, 4 * blk + 4):
                        fil.add(lambda st=st: emit_xt(st), tag=f"xt{st}")
                        fil.add(lambda st=st: emit_v(st), tag=f"v{st}")
                    fil.add(lambda blk=blk: emit_kq(0, blk, 0))
                    fil.add(lambda blk=blk: emit_kq(0, blk, 1),
                            tag=f"kq0s{blk}")
                for g in range(1, NG):
                    for sg in range(4):
                        fil.add(lambda g=g, sg=sg: emit_kq(g, sg, 0))
                        fil.add(lambda g=g, sg=sg: emit_kq(g, sg, 1))
                    fil.add(lambda: None, tag=f"pair{g}")
                for qcf in (1, 2, 3, 0):
                    for jt in range(qcf * QCT, (qcf + 1) * QCT):
                        fil.add(lambda jt=jt: emit_fc(jt), tag=f"fc{jt}",
                                gate=f"cat_qc{jt // QCT}")

                # ---- attention (exp-paced), filler in the gaps ---------
                for g in range(NG):
                    if g > 0:
                        fil.drain(f"pair{g}")
                    qcs = (1, 2, 3, 0) if g == NG - 1 else range(NQC)
                    for qc in qcs:
                        if g == 0 and qc > 0:
                            fil.drain(f"kq0s{qc}")
                        ks = chunk_ks[qc]
                        vjm = chunk_vj[qc]
                        if not ks:
                            continue
                        union = sorted({j for v in vjm.values() for j in v})
                        fast = vjm[ks[0]] == union
                        po = ppo.tile([HD + 1, 2 * QCW], F32, tag="po")
                        if not fast:
                            nc.vector.memset(po[:, :], 0.0)
                        nks = len(ks)

                        def emit_av(k, idx, runs, pt):
                            for h in range(2):
                                for ja, jb in runs:
                                    nc.tensor.matmul(
                                        po[0:HD + 1,
                                           h * QCW + ja * ST:
                                           h * QCW + (jb + 1) * ST],
                                        vb[:, k, 2 * g + h, :],
                                        pt[:, h * QCW + ja * ST:
                                           h * QCW + (jb + 1) * ST],
                                        start=(fast and idx == 0),
                                        stop=(fast and idx == nks - 1),
                                        skip_group_check=True)

                        pending = None
                        for idx, k in enumerate(ks):
                            runs = _runs(vjm[k])
                            psc = ppsc.tile([ST, 2 * QCW], F32, tag="psc")
                            for ja, jb in runs:
                                for h in range(2):
                                    nc.tensor.matmul(
                                        psc[:, h * QCW + ja * ST:
                                            h * QCW + (jb + 1) * ST],
                                        kt[g][h * HD:(h + 1) * HD,
                                              k * ST:(k + 1) * ST],
                                        qt[g][h * HD:(h + 1) * HD,
                                              qc * QCW + ja * ST:
                                              qc * QCW + (jb + 1) * ST],
                                        start=True, stop=True)
                            if pending is not None:
                                emit_av(*pending)
                            pt = ppt.tile([ST, 2 * QCW], BF16, tag="pt")
                            nc.scalar.activation(
                                pt[:, :], psc[:, :],
                                mybir.ActivationFunctionType.Exp,
                                scale=1.0 / float(np.sqrt(HD)))
                            for j in vjm[k]:
                                if cls[qc * QCT + j, k] == 2:
                                    m = mixed_idx[(qc * QCT + j, k)]
                                    for h in range(2):
                                        nc.vector.tensor_mul(
                                            pt[:, h * QCW + j * ST:
                                               h * QCW + (j + 1) * ST],
                                            pt[:, h * QCW + j * ST:
                                               h * QCW + (j + 1) * ST],
                                            mtb[:, m, :])
                            pending = (k, idx, runs, pt)
                            fil.pull(1)
                        emit_av(*pending)
                        # finalize (g, qc): free po via a DVE copy, then
                        # normalize out of SBUF.
                        sfin = pfin.tile([HD + 1, 2 * QCW], F32, tag="sfin")
                        nc.vector.tensor_copy(sfin[:, :], po[:, :])
                        ltmp = pfin.tile([1, 2 * QCW], F32, tag="ltmp")
                        nc.vector.tensor_copy(ltmp[:, :], sfin[HD:HD + 1, :])
                        rec = pfin.tile([1, 2 * QCW], F32, tag="rec")
                        nc.vector.reciprocal_approx_fast(rec[:, :], ltmp[:, :])
                        rbs = pfin.tile([HD, 2 * QCW], F32, tag="rbs")
                        nc.gpsimd.partition_broadcast(
                            rbs[:, :], rec[0:1, :], channels=HD)
                        for h in range(2):
                            nc.vector.tensor_mul(
                                cat[g][h * HD:(h + 1) * HD,
                                       qc * QCW:(qc + 1) * QCW],
                                sfin[0:HD, h * QCW:(h + 1) * QCW],
                                rbs[:, h * QCW:(h + 1) * QCW])
                        if g == NG - 1:
                            fil.open_gate(f"cat_qc{qc}")
                fil.drain_all()
            pxb_cm.__exit__(None, None, None)

    nc.compile()
    return nc


_CACHE = {}
LAST_RESULT = None


def _get_program(mask):
    key = mask.tobytes()
    if key not in _CACHE:
        cls, mixed = _classify(mask)
        _CACHE[key] = (_build(cls, mixed, max(len(mixed), 1)), cls, mixed)
    return _CACHE[key]


def kernel(x, mask, Wq, bq, Wk, bk, Wv, bv, Wo, bo):
    x = np.asarray(x, dtype=np.float32)
    mask = np.asarray(mask)
    Wq = np.asarray(Wq, dtype=np.float32)
    Wk = np.asarray(Wk, dtype=np.float32)
    Wv = np.asarray(Wv, dtype=np.float32)
    Wo = np.asarray(Wo, dtype=np.float32)
    nc, cls, mixed = _get_program(mask)

    n_maskt = max(len(mixed), 1)
    mt = np.zeros((n_maskt, ST, ST), dtype=BF)
    for i, (j, k) in enumerate(mixed):
        blk = mask[j * ST:(j + 1) * ST, k * ST:(k + 1) * ST]
        mt[i] = (blk != 0).T.astype(BF)

    def pack_pair(W, hh):
        # [128, NDC, NG, 128]: [p, c, g, m*64+e] = W[8hh + 2g+m, 128c+p, e]
        Wl = W[hh * HL:(hh + 1) * HL].reshape(NG, 2, NDC, ST, HD)
        return np.ascontiguousarray(
            Wl.transpose(3, 2, 0, 1, 4).reshape(ST, NDC, NG, ST).astype(BF))

    in_maps = []
    for c in range(N_CORES):
        b, hh = c // 2, c % 2
        Wvl = Wv[hh * HL:(hh + 1) * HL].reshape(HL, NDC, ST, HD)
        wvb = np.ascontiguousarray(
            Wvl.transpose(2, 1, 0, 3).reshape(ST, NDC, HL * HD).astype(BF))
        Wol = Wo[hh * HL * HD:(hh + 1) * HL * HD].reshape(NG, 2, HD, D)
        wob = np.ascontiguousarray(
            Wol.transpose(1, 2, 0, 3).reshape(ST, NG, D).astype(BF))
        bql = np.asarray(bq, dtype=np.float32)[hh * HL:(hh + 1) * HL]
        bkl = np.asarray(bk, dtype=np.float32)[hh * HL:(hh + 1) * HL]
        bqp = np.ascontiguousarray(
            bql.reshape(NG, 2, HD).transpose(1, 2, 0).reshape(ST, NG))
        bkp = np.ascontiguousarray(
            bkl.reshape(NG, 2, HD).transpose(1, 2, 0).reshape(ST, NG))
        m = {
            "x": np.ascontiguousarray(x[b].astype(BF)),
            "wqp": pack_pair(Wq, hh),
            "wkp": pack_pair(Wk, hh),
            "wvb": wvb,
            "wob": wob,
            "bqp": bqp,
            "bkp": bkp,
            "bvf": np.ascontiguousarray(
                np.asarray(bv, dtype=np.float32)[hh * HL:(hh + 1) * HL]),
            "bob": (np.asarray(bo, dtype=np.float32) if hh == 0
                    else np.zeros(D, dtype=np.float32)),
            "maskt": mt,
        }
        in_maps.append(m)

    res = run_bass_kernel_spmd(
        nc, in_maps, core_ids=list(range(N_CORES)),
        trace=os.environ.get("BASS_KERNEL_TRACE", "0") == "1")
    global LAST_RESULT
    LAST_RESULT = res

    out = np.empty((B, S, D), dtype=np.float32)
    for b in range(B):
        out[b] = (res.results[2 * b]["out"].astype(np.float32)
                  + res.results[2 * b + 1]["out"].astype(np.float32))
    return out


# revision 24
# speedup vs baseline: 1.2136x; 1.0049x over previous
"""Trainium2 Bass kernel: causal multi-head attention (B=4,S=2048,D=1024,H=16).

Sharding (8 cores, host-side pair reduction): core c -> batch b=c//2,
head-half hh=c%2 (local heads hh*8..hh*8+7, i.e. 4 head pairs).  Each core
computes Q/K/V for its 8 heads over ALL 2048 rows, full causal attention,
and a PARTIAL fc_out against the row-shard Wo[hh*512:(hh+1)*512].  The host
sums the two partials per batch (the "all-reduce" of the row-sharded Wo).

Device pipeline per core (all matmuls bf16, f32 accumulation):
  - x^T arrives directly via 8 DMA-xbar transposes from DRAM (no PE
    transposes, no row-major staging).
  - Attention (the ScalarE exp stream is the pacer): per pair g, per
    q-chunk of 512 cols, per k-tile: scores^T pair = two row-tiled
    concurrent matmuls (heads at array rows 0-63 / 64-127) -> one
    1024-wide exp on ScalarE (scale folded, PSUM->SBUF bf16), 0/1 mask
    multiply on diag blocks, ones-augmented AV accumulation one k-step
    behind (row 64 = softmax denominator).
  - All other PE work (V projections, K^T/Q^T of later pairs, fc_out
    tiles) is emitted as "filler" chunks pulled into the exp-wait gaps,
    gated by markers so the in-order PE queue can never deadlock.
  - Finalize per (g, q-chunk): free po via a DVE copy, reciprocal of the
    denominators, GpSimd partition-broadcast, normalize into cat (bf16).
  - fc_out tiles run as filler during the last pair; bf16 output.

Weights are pre-packed on the host into the exact stationary layouts
(bf16).  The program is specialized at build time to the mask's 128x128
block structure (computed from the actual mask input, so it stays correct
for any mask).
"""

import os
import numpy as np
import ml_dtypes

import concourse.bass as bass
import concourse.mybir as mybir
import concourse.tile as tile
from concourse import bacc
from concourse.bass_utils import run_bass_kernel_spmd
from concourse.masks import make_identity

B, S, D, H, HD = 4, 2048, 1024, 16, 64
N_CORES = 8
ST = 128                 # tile edge
NKT = S // ST            # 16 k tiles
NQT = S // ST            # 16 q tiles
NDC = D // ST            # 8 contraction chunks
HL = H // 2              # 8 local heads per core
NG = HL // 2             # 4 local head pairs
NQC = 4                  # q chunks per core
QCW = S // NQC           # 512 cols per q chunk (4 q tiles)
QCT = QCW // ST          # 4 q tiles per chunk

F32 = mybir.dt.float32
BF16 = mybir.dt.bfloat16
BF = ml_dtypes.bfloat16


def _classify(mask: np.ndarray):
    """128x128 block structure of the mask: 0 skip, 1 full, 2 mixed."""
    cls = np.zeros((NQT, NKT), dtype=int)
    for j in range(NQT):
        for k in range(NKT):
            blk = mask[j * ST:(j + 1) * ST, k * ST:(k + 1) * ST]
            if (blk != 0).all():
                cls[j, k] = 1
            elif (blk == 0).all():
                cls[j, k] = 0
            else:
                cls[j, k] = 2
    mixed = [(j, k) for j in range(NQT) for k in range(NKT) if cls[j, k] == 2]
    return cls, mixed


def _runs(valid):
    """Contiguous runs [(ja, jb)] of a sorted list of chunk-local j."""
    runs = []
    for j in valid:
        if runs and j == runs[-1][1] + 1:
            runs[-1][1] = j
        else:
            runs.append([j, j])
    return [(a, b) for a, b in runs]


class Filler:
    """Ordered queue of PE-work chunks with tags (drain points) and gates."""

    def __init__(self):
        self.q = []           # (tag, gate, fn)
        self.open = set()
        self.emitted = set()

    def add(self, fn, tag=None, gate=None):
        self.q.append((tag, gate, fn))

    def open_gate(self, gate):
        self.open.add(gate)

    def _emit_front(self):
        tag, gate, fn = self.q.pop(0)
        fn()
        if tag:
            self.emitted.add(tag)
        return tag

    def pull(self, n=1):
        for _ in range(n):
            if not self.q:
                return
            tag, gate, fn = self.q[0]
            if gate is not None and gate not in self.open:
                return
            self._emit_front()

    def drain(self, tag):
        if tag in self.emitted:
            return
        while self.q:
            g = self.q[0][1]
            assert g is None or g in self.open, f"drain past closed gate {g}"
            if self._emit_front() == tag:
                return
        raise KeyError(tag)

    def drain_all(self):
        while self.q:
            self._emit_front()


def _build(cls, mixed, n_maskt):
    nc = bacc.Bacc("TRN2", target_bir_lowering=False, debug=False,
                   num_devices=N_CORES)

    x_d = nc.dram_tensor("x", [S, D], BF16, kind="ExternalInput")
    wqp_d = nc.dram_tensor("wqp", [ST, NDC, NG, ST], BF16, kind="ExternalInput")
    wkp_d = nc.dram_tensor("wkp", [ST, NDC, NG, ST], BF16, kind="ExternalInput")
    wvb_d = nc.dram_tensor("wvb", [ST, NDC, HL * HD], BF16, kind="ExternalInput")
    wob_d = nc.dram_tensor("wob", [ST, NG, D], BF16, kind="ExternalInput")
    bqp_d = nc.dram_tensor("bqp", [ST, NG], F32, kind="ExternalInput")
    bkp_d = nc.dram_tensor("bkp", [ST, NG], F32, kind="ExternalInput")
    bvf_d = nc.dram_tensor("bvf", [HL, HD], F32, kind="ExternalInput")
    bob_d = nc.dram_tensor("bob", [D], F32, kind="ExternalInput")
    mt_d = nc.dram_tensor("maskt", [n_maskt, ST, ST], BF16, kind="ExternalInput")
    out_d = nc.dram_tensor("out", [S, D], BF16, kind="ExternalOutput")

    mixed_idx = {jk: i for i, jk in enumerate(mixed)}

    chunk_ks, chunk_vj = [], []
    for qc in range(NQC):
        vj = {}
        for k in range(NKT):
            v = [j for j in range(QCT) if cls[qc * QCT + j, k]]
            if v:
                vj[k] = v
        chunk_ks.append(sorted(vj))
        chunk_vj.append(vj)

    with tile.TileContext(nc) as tc:
        with tc.tile_pool(name="pp", bufs=1) as pp:
            # ---- persistent SBUF ----------------------------------------
            kt = [pp.tile([ST, S], BF16, name=f"kt{g}", tag=f"kt{g}")
                  for g in range(NG)]
            qt = [pp.tile([ST, S], BF16, name=f"qt{g}", tag=f"qt{g}")
                  for g in range(NG)]
            cat = [pp.tile([ST, S], BF16, name=f"cat{g}", tag=f"cat{g}")
                   for g in range(NG)]
            xt = [pp.tile([ST, S], BF16, name=f"xt{c}", tag=f"xt{c}")
                  for c in range(NDC)]
            vb = pp.tile([ST, NKT, HL, HD + 1], BF16, name="vb", tag="vb")
            wqp = pp.tile([ST, NDC, NG, ST], BF16, name="wqp", tag="wqp")
            wkp = pp.tile([ST, NDC, NG, ST], BF16, name="wkp", tag="wkp")
            wvb = pp.tile([ST, NDC, HL * HD], BF16, name="wvb", tag="wvb")
            wob = pp.tile([ST, NG, D], BF16, name="wob", tag="wob")
            bqp = pp.tile([ST, NG], F32, name="bqp", tag="bqp")
            bkp = pp.tile([ST, NG], F32, name="bkp", tag="bkp")
            bvf = pp.tile([ST, HL, HD], F32, name="bvf", tag="bvf")
            bob = pp.tile([ST, D], F32, name="bob", tag="bob")
            mtb = pp.tile([ST, max(n_maskt, 1), ST], BF16, name="mtb", tag="mtb")
            ident = pp.tile([ST, ST], BF16, name="ident", tag="ident")

            # prefetch the exp ACT table set during the DMA-bound ramp so
            # the first real exp doesn't pay the ~2.7us table load.
            actwarm = pp.tile([1, 16], F32, name="actwarm", tag="actwarm")
            nc.vector.memset(actwarm[:, :], 0.0)
            nc.scalar.activation(actwarm[:, :], actwarm[:, :],
                                 mybir.ActivationFunctionType.Exp)
            make_identity(nc, ident[:, :])
            # weights on the gpsimd (SWDGE) queue, most-urgent first
            nc.gpsimd.dma_start(wkp[:, :, :, :], wkp_d.ap())
            nc.gpsimd.dma_start(wvb[:, :, :], wvb_d.ap())
            nc.gpsimd.dma_start(wqp[:, :, :, :], wqp_d.ap())
            nc.gpsimd.dma_start(mtb[:, :, :],
                                mt_d.ap().rearrange("m p f -> p m f"))
            nc.gpsimd.dma_start(wob[:, :, :], wob_d.ap())
            # small tensors after the x chunks on the HWDGE queues
            nc.scalar.dma_start(bqp[:, :], bqp_d.ap())
            nc.scalar.dma_start(bkp[:, :], bkp_d.ap())
            src = bvf_d.ap()
            nc.scalar.dma_start(
                bvf[:, :, :],
                bass.AP(tensor=src.tensor, offset=src.offset,
                        ap=[[0, ST]] + list(src.ap)))
            src = bob_d.ap()
            nc.scalar.dma_start(
                bob[:, :],
                bass.AP(tensor=src.tensor, offset=src.offset,
                        ap=[[0, ST]] + list(src.ap)))

            nc.vector.memset(vb[:, :, :, HD:HD + 1], 1.0)

            # ---- ramp: x^T(st0-3) via PE transposes, K0/Q0 sg0, V st0-3
            pxb_cm = tc.tile_pool(name="pxb", bufs=4)
            pxb = pxb_cm.__enter__()
            with (
                tc.tile_pool(name="ppst", bufs=3, space="PSUM") as ppst,
                tc.tile_pool(name="ppvr", bufs=2, space="PSUM") as ppvr,
            ):
                def emit_v_ramp(st):
                    psv = ppvr.tile([ST, HL * HD], F32, tag="pvr")
                    for c in range(NDC):
                        nc.tensor.matmul(
                            psv[:, :], xt[c][:, st * ST:(st + 1) * ST],
                            wvb[:, c, :], start=(c == 0), stop=(c == NDC - 1),
                            skip_group_check=True)
                    nc.vector.tensor_add(
                        vb[:, st, :, 0:HD],
                        psv[:, :].rearrange("p (h e) -> p h e", h=HL),
                        bvf[:, :, :])

                def emit_kq_ramp(g, sg, which):
                    w_t, bias_t, dst = ((wkp, bkp, kt[g]) if which == 0
                                        else (wqp, bqp, qt[g]))
                    ps = ppvr.tile([ST, 512], F32, tag="pvr")
                    for c in range(NDC):
                        nc.tensor.matmul(
                            ps[:, :], w_t[:, c, g, :],
                            xt[c][:, sg * 512:(sg + 1) * 512],
                            start=(c == 0), stop=(c == NDC - 1),
                            skip_group_check=True)
                    nc.vector.tensor_scalar(
                        dst[:, sg * 512:(sg + 1) * 512], ps[:, :],
                        bias_t[:, g:g + 1], None, mybir.AluOpType.add)
                # (sg0 of pair 0 is emitted here in the ramp)

                for st in range(4):
                    xb = pxb.tile([ST, D], BF16, tag="xb")
                    eng = nc.sync if st % 2 == 0 else nc.scalar
                    eng.dma_start(xb[:, :],
                                  x_d.ap()[st * ST:(st + 1) * ST, :])
                    for c in range(NDC):
                        pst = ppst.tile([ST, ST], BF16, tag="pst")
                        nc.tensor.transpose(
                            pst[:, :], xb[:, c * ST:(c + 1) * ST], ident[:, :])
                        nc.scalar.copy(xt[c][:, st * ST:(st + 1) * ST],
                                       pst[:, :])
                emit_kq_ramp(0, 0, 0)
                emit_kq_ramp(0, 0, 1)
                for s0 in range(4):
                    emit_v_ramp(s0)

            with (
                tc.tile_pool(name="ppsc", bufs=2, space="PSUM") as ppsc,
                tc.tile_pool(name="ppo", bufs=1, space="PSUM") as ppo,
                tc.tile_pool(name="ppv", bufs=2, space="PSUM") as ppv,
                tc.tile_pool(name="ppt", bufs=4) as ppt,
                tc.tile_pool(name="pfin", bufs=2) as pfin,
                tc.tile_pool(name="pfcs", bufs=3) as pfcs,
            ):
                def emit_xt(st):
                    xb = pxb.tile([ST, D], BF16, tag="xb")
                    eng = nc.sync if st % 2 == 0 else nc.scalar
                    eng.dma_start(xb[:, :],
                                  x_d.ap()[st * ST:(st + 1) * ST, :])
                    for c in range(NDC):
                        pst = ppv.tile([ST, ST], BF16, tag="pv", name="pst")
                        nc.tensor.transpose(
                            pst[:, :], xb[:, c * ST:(c + 1) * ST], ident[:, :])
                        nc.vector.tensor_copy(
                            xt[c][:, st * ST:(st + 1) * ST], pst[:, :])

                def emit_v(st):
                    psv = ppv.tile([ST, HL * HD], F32, tag="pv")
                    for c in range(NDC):
                        nc.tensor.matmul(
                            psv[:, :], xt[c][:, st * ST:(st + 1) * ST],
                            wvb[:, c, :], start=(c == 0), stop=(c == NDC - 1),
                            skip_group_check=True)
                    nc.vector.tensor_add(
                        vb[:, st, :, 0:HD],
                        psv[:, :].rearrange("p (h e) -> p h e", h=HL),
                        bvf[:, :, :])

                def emit_kq(g, sg, which):
                    w_t, bias_t, dst = ((wkp, bkp, kt[g]) if which == 0
                                        else (wqp, bqp, qt[g]))
                    ps = ppv.tile([ST, 512], F32, tag="pv")
                    for c in range(NDC):
                        nc.tensor.matmul(
                            ps[:, :], w_t[:, c, g, :],
                            xt[c][:, sg * 512:(sg + 1) * 512],
                            start=(c == 0), stop=(c == NDC - 1),
                            skip_group_check=True)
                    nc.vector.tensor_scalar(
                        dst[:, sg * 512:(sg + 1) * 512], ps[:, :],
                        bias_t[:, g:g + 1], None, mybir.AluOpType.add)

                def emit_fc(jt):
                    py = [ppv.tile([ST, 512], F32, tag="pv", name=f"py{n}")
                          for n in range(2)]
                    for g in range(NG):
                        for n in range(2):
                            nc.tensor.matmul(
                                py[n][:, :],
                                cat[g][:, jt * ST:(jt + 1) * ST],
                                wob[:, g, n * 512:(n + 1) * 512],
                                start=(g == 0), stop=(g == NG - 1),
                                skip_group_check=True)
                    ysb = pfcs.tile([ST, D], BF16, tag="ysb")
                    for n in range(2):
                        nc.vector.tensor_add(ysb[:, n * 512:(n + 1) * 512],
                                             py[n][:, :],
                                             bob[:, n * 512:(n + 1) * 512])
                    eng = nc.sync if jt % 2 == 0 else nc.scalar
                    eng.dma_start(out_d.ap()[jt * ST:(jt + 1) * ST, :],
                                  ysb[:, :])

                # ---- filler queue --------------------------------------
                fil = Filler()
                for blk in range(1, 4):
                    for st in range(4 * blk, 4 * blk + 4):
                        fil.add(lambda st=st: emit_xt(st), tag=f"xt{st}")
                        fil.add(lambda st=st: emit_v(st), tag=f"v{st}")
                    fil.add(lambda blk=blk: emit_kq(0, blk, 0))
                    fil.add(lambda blk=blk: emit_kq(0, blk, 1),
                            tag=f"kq0s{blk}")
                for g in range(1, NG):
                    for sg in range(4):
                        fil.add(lambda g=g, sg=sg: emit_kq(g, sg, 0))
                        fil.add(lambda g=g, sg=sg: emit_kq(g, sg, 1))
                    fil.add(lambda: None, tag=f"pair{g}")
                for qcf in (1, 2, 3, 0):
                    for jt in range(qcf * QCT, (qcf + 1) * QCT):
                        fil.add(lambda jt=jt: emit_fc(jt), tag=f"fc{jt}",
                                gate=f"cat_qc{jt // QCT}")

                # ---- attention (exp-paced), filler in the gaps ---------
                for g in range(NG):
                    if g > 0:
                        fil.drain(f"pair{g}")
                    qcs = (1, 2, 3, 0) if g == NG - 1 else range(NQC)
                    for qc in qcs:
                        if g == 0 and qc > 0:
                            fil.drain(f"kq0s{qc}")
                        ks = chunk_ks[qc]
                        vjm = chunk_vj[qc]
                        if not ks:
                            continue
                        union = sorted({j for v in vjm.values() for j in v})
                        fast = vjm[ks[0]] == union
                        po = ppo.tile([HD + 1, 2 * QCW], F32, tag="po")
                        if not fast:
                            nc.vector.memset(po[:, :], 0.0)
                        nks = len(ks)

                        def emit_av(k, idx, runs, pt):
                            for h in range(2):
                                for ja, jb in runs:
                                    nc.tensor.matmul(
                                        po[0:HD + 1,
                                           h * QCW + ja * ST:
                                           h * QCW + (jb + 1) * ST],
                                        vb[:, k, 2 * g + h, :],
                                        pt[:, h * QCW + ja * ST:
                                           h * QCW + (jb + 1) * ST],
                                        start=(fast and idx == 0),
                                        stop=(fast and idx == nks - 1),
                                        skip_group_check=True)

                        pending = None
                        for idx, k in enumerate(ks):
                            runs = _runs(vjm[k])
                            psc = ppsc.tile([ST, 2 * QCW], F32, tag="psc")
                            for ja, jb in runs:
                                for h in range(2):
                                    nc.tensor.matmul(
                                        psc[:, h * QCW + ja * ST:
                                            h * QCW + (jb + 1) * ST],
                                        kt[g][h * HD:(h + 1) * HD,
                                              k * ST:(k + 1) * ST],
                                        qt[g][h * HD:(h + 1) * HD,
                                              qc * QCW + ja * ST:
                                              qc * QCW + (jb + 1) * ST],
                                        start=True, stop=True)
                            if pending is not None:
                                emit_av(*pending)
                            pt = ppt.tile([ST, 2 * QCW], BF16, tag="pt")
                            nc.scalar.activation(
                                pt[:, :], psc[:, :],
                                mybir.ActivationFunctionType.Exp,
                                scale=1.0 / float(np.sqrt(HD)))
                            for j in vjm[k]:
                                if cls[qc * QCT + j, k] == 2:
                                    m = mixed_idx[(qc * QCT + j, k)]
                                    for h in range(2):
                                        nc.vector.tensor_mul(
                                            pt[:, h * QCW + j * ST:
                                               h * QCW + (j + 1) * ST],
                                            pt[:, h * QCW + j * ST:
                                               h * QCW + (j + 1) * ST],
                                            mtb[:, m, :])
                            pending = (k, idx, runs, pt)
                            fil.pull(1)
                        emit_av(*pending)
                        # finalize (g, qc): free po via a DVE copy, then
                        # normalize out of SBUF.
                        sfin = pfin.tile([HD + 1, 2 * QCW], F32, tag="sfin")
                        nc.vector.tensor_copy(sfin[:, :], po[:, :])
                        ltmp = pfin.tile([1, 2 * QCW], F32, tag="ltmp")
                        nc.vector.tensor_copy(ltmp[:, :], sfin[HD:HD + 1, :])
                        rec = pfin.tile([1, 2 * QCW], F32, tag="rec")
                        nc.vector.reciprocal_approx_fast(rec[:, :], ltmp[:, :])
                        rbs = pfin.tile([HD, 2 * QCW], F32, tag="rbs")
                        nc.gpsimd.partition_broadcast(
                            rbs[:, :], rec[0:1, :], channels=HD)
                        for h in range(2):
                            nc.vector.tensor_mul(
                                cat[g][h * HD:(h + 1) * HD,
                                       qc * QCW:(qc + 1) * QCW],
                                sfin[0:HD, h * QCW:(h + 1) * QCW],
                                rbs[:, h * QCW:(h + 1) * QCW])
                        if g == NG - 1:
                            fil.open_gate(f"cat_qc{qc}")
                fil.drain_all()
            pxb_cm.__exit__(None, None, None)

    nc.compile()
    return nc


_CACHE = {}
LAST_RESULT = None


def _get_program(mask):
    key = mask.tobytes()
    if key not in _CACHE:
        cls, mixed = _classify(mask)
        _CACHE[key] = (_build(cls, mixed, max(len(mixed), 1)), cls, mixed)
    return _CACHE[key]


def kernel(x, mask, Wq, bq, Wk, bk, Wv, bv, Wo, bo):
    x = np.asarray(x, dtype=np.float32)
    mask = np.asarray(mask)
    Wq = np.asarray(Wq, dtype=np.float32)
    Wk = np.asarray(Wk, dtype=np.float32)
    Wv = np.asarray(Wv, dtype=np.float32)
    Wo = np.asarray(Wo, dtype=np.float32)
    nc, cls, mixed = _get_program(mask)

    n_maskt = max(len(mixed), 1)
    mt = np.zeros((n_maskt, ST, ST), dtype=BF)
    for i, (j, k) in enumerate(mixed):
        blk = mask[j * ST:(j + 1) * ST, k * ST:(k + 1) * ST]
        mt[i] = (blk != 0).T.astype(BF)

    def pack_pair(W, hh):
        # [128, NDC, NG, 128]: [p, c, g, m*64+e] = W[8hh + 2g+m, 128c+p, e]
        Wl = W[hh * HL:(hh + 1) * HL].reshape(NG, 2, NDC, ST, HD)
        return np.ascontiguousarray(
            Wl.transpose(3, 2, 0, 1, 4).reshape(ST, NDC, NG, ST).astype(BF))

    in_maps = []
    for c in range(N_CORES):
        b, hh = c // 2, c % 2
        Wvl = Wv[hh * HL:(hh + 1) * HL].reshape(HL, NDC, ST, HD)
        wvb = np.ascontiguousarray(
            Wvl.transpose(2, 1, 0, 3).reshape(ST, NDC, HL * HD).astype(BF))
        Wol = Wo[hh * HL * HD:(hh + 1) * HL * HD].reshape(NG, 2, HD, D)
        wob = np.ascontiguousarray(
            Wol.transpose(1, 2, 0, 3).reshape(ST, NG, D).astype(BF))
        bql = np.asarray(bq, dtype=np.float32)[hh * HL:(hh + 1) * HL]
        bkl = np.asarray(bk, dtype=np.float32)[hh * HL:(hh + 1) * HL]
        bqp = np.ascontiguousarray(
            bql.reshape(NG, 2, HD).transpose(1, 2, 0).reshape(ST, NG))
        bkp = np.ascontiguousarray(
            bkl.reshape(NG, 2, HD).transpose(1, 2, 0).reshape(ST, NG))
        m = {
            "x": np.ascontiguousarray(x[b].astype(BF)),
            "wqp": pack_pair(Wq, hh),
            "wkp": pack_pair(Wk, hh),
            "wvb": wvb,
            "wob": wob,
            "bqp": bqp,
            "bkp": bkp,
            "bvf": np.ascontiguousarray(
                np.asarray(bv, dtype=np.float32)[hh * HL:(hh + 1) * HL]),
            "bob": (np.asarray(bo, dtype=np.float32) if hh == 0
                    else np.zeros(D, dtype=np.float32)),
            "maskt": mt,
        }
        in_maps.append(m)

    res = run_bass_kernel_spmd(
        nc, in_maps, core_ids=list(range(N_CORES)),
        trace=os.environ.get("BASS_KERNEL_TRACE", "0") == "1")
    global LAST_RESULT
    LAST_RESULT = res

    out = np.empty((B, S, D), dtype=np.float32)
    for b in range(B):
        out[b] = (res.results[2 * b]["out"].astype(np.float32)
                  + res.results[2 * b + 1]["out"].astype(np.float32))
    return out
